# revision 2
# baseline (speedup 1.0000x reference)
"""Trainium2 Bass kernel for nn_CESAR_24309514895978 (ragged_sequence).

Math (per batch b):
  m0 = (attention_masks==1)&(token_type_ids==0); m1 = (attention_masks==1)&(token_type_ids==1)
  score[i,j] = |emb_n[i]. emb_n[j]|  (L2-normalized embeddings)
  logits[i,j] = (emb@Wq.T+bq)[i] . (emb@Wk.T+bk)[j]
  cs[b] = sum_{valid ij} softmax_flat(logits | pair_mask)[i,j] * score[i,j]

Device computes, per batch (data-parallel: 2 batches per core x 8 cores):
  - rsq[j] = sum_d emb[j,d]^2 via bf16 squares + ones-matmul; r = 1/sqrt(rsq)
  - QT/KT = Wq/Wk projections (fp32r matmuls, PSUM fp32, bias via ACT)
  - L[i,j] = QT.T@KT + (-1e30 masks via a K=2 static matmul row-pair)
  - M = masked max (DVE reduce + tiny transpose DMA + DRAM-roundtrip broadcast)
  - E = exp(L - M)  (ACT, accum_out -> Z partial sums)
  - W partials = sum_j E * |G| * r_j  (G = gram matmul; stt fused mult/mult/accum)
Host: r_i scaling + final sums + W/Z division (tiny), plus input layout/rounding.
"""
import numpy as np

import concourse.tile as tile
from concourse import bacc, mybir
from concourse.bass_utils import run_bass_kernel_spmd

B, S, D = 16, 512, 1024
NCORES = 8
BPC = B // NCORES          # batches per core
NCH = D // 128             # 8 contraction chunks
NIC = S // 128             # 4 i-chunks
NEG = np.float32(-1e30)

F32 = mybir.dt.float32
F32R = mybir.dt.float32r
BF16 = mybir.dt.bfloat16
AFT = mybir.ActivationFunctionType
ALU = mybir.AluOpType
AX = mybir.AxisListType

PROFILE = False            # set True (e.g. from test.py) to capture NTFF profile
LAST_RESULTS = None        # BassKernelResults of the last run (for test.py)

_built = None


def _to_fp32r(x: np.ndarray) -> np.ndarray:
    """Round fp32 -> fp32r encoding (RNE to 11 explicit mantissa bits)."""
    u = np.ascontiguousarray(x, dtype=np.float32).view(np.uint32).astype(np.uint64)
    u = (u + 0x7FF + ((u >> 12) & 1)) & np.uint64(0xFFFFF000)
    return u.astype(np.uint32).view(np.float32)


def _build():
    global _built
    if _built is not None:
        return _built

    nc = bacc.Bacc("TRN2", target_bir_lowering=False, debug=False)

    embT_d = nc.dram_tensor("embT", [BPC, NCH, 128, S], F32R, kind="ExternalInput").ap()
    wqT_d = nc.dram_tensor("wqT", [NCH, 128, D], F32R, kind="ExternalInput").ap()
    wkT_d = nc.dram_tensor("wkT", [NCH, 128, D], F32R, kind="ExternalInput").ap()
    bqc_d = nc.dram_tensor("bqc", [128, NCH], F32, kind="ExternalInput").ap()
    bkc_d = nc.dram_tensor("bkc", [128, NCH], F32, kind="ExternalInput").ap()
    lrows_d = nc.dram_tensor("lrows", [BPC, 2, S], F32R, kind="ExternalInput").ap()
    rrows_d = nc.dram_tensor("rrows", [BPC, 2, S], F32R, kind="ExternalInput").ap()

    zw_d = nc.dram_tensor("zw", [BPC, 2, 128, NIC], F32, kind="ExternalOutput").ap()
    rout_d = nc.dram_tensor("rout", [BPC, S], F32, kind="ExternalOutput").ap()

    negm_s = nc.dram_tensor("negm_scratch", [BPC, 1], F32).ap()
    w2_s = nc.dram_tensor("w2_scratch", [BPC, S], F32).ap()

    with tile.TileContext(nc) as tc:
        with (
            tc.tile_pool(name="wpool", bufs=16) as wpool,
            tc.tile_pool(name="spool", bufs=1) as spool,
            tc.tile_pool(name="epool", bufs=16) as epool,
            tc.tile_pool(name="sqpool", bufs=3) as sqpool,
            tc.tile_pool(name="qkpool", bufs=16) as qkpool,
            tc.tile_pool(name="w2pool", bufs=2) as w2pool,
            tc.tile_pool(name="gapool", bufs=2) as gapool,
            tc.tile_pool(name="gwpool", bufs=4) as gwpool,
            tc.tile_pool(name="Epool", bufs=2) as Epool,
            tc.tile_pool(name="scrpool", bufs=1) as scrpool,
            tc.tile_pool(name="tiny", bufs=2) as tiny,
            tc.tile_pool(name="lrpool", bufs=2) as lrpool,
            tc.tile_pool(name="psP", bufs=4, space="PSUM") as psP,
            tc.tile_pool(name="psL", bufs=4, space="PSUM") as psL,
        ):
            wq_t, wk_t = [], []
            for c in range(NCH):
                t = wpool.tile([128, D], F32R, tag="w")
                nc.sync.dma_start(out=t[:], in_=wqT_d[c])
                wq_t.append(t)
            for c in range(NCH):
                t = wpool.tile([128, D], F32R, tag="w")
                nc.sync.dma_start(out=t[:], in_=wkT_d[c])
                wk_t.append(t)
            bqc_t = spool.tile([128, NCH], F32, tag="bqc")
            nc.sync.dma_start(out=bqc_t[:], in_=bqc_d)
            bkc_t = spool.tile([128, NCH], F32, tag="bkc")
            nc.sync.dma_start(out=bkc_t[:], in_=bkc_d)
            ones_bf = spool.tile([128, 1], BF16, tag="ones_bf")
            nc.vector.memset(ones_bf[:], 1.0)

            for b in range(BPC):
                # ---- load inputs for this batch
                emb_t = []
                for c in range(NCH):
                    t = epool.tile([128, S], F32R, tag="emb")
                    nc.sync.dma_start(out=t[:], in_=embT_d[b, c])
                    emb_t.append(t)
                lr_t = lrpool.tile([2, S], F32R, tag="lr")
                nc.sync.dma_start(out=lr_t[:], in_=lrows_d[b])
                rr_t = lrpool.tile([2, S], F32R, tag="rr")
                nc.sync.dma_start(out=rr_t[:], in_=rrows_d[b])

                # ---- rsq / r / W2 (row of 1/||e_j||, replicated to 128 partitions)
                rsq_ps = psL.tile([1, S], F32, tag="Lps")
                for c in range(NCH):
                    sq = sqpool.tile([128, S], BF16, tag="sq")
                    nc.vector.tensor_mul(sq[:], emb_t[c][:].bitcast(F32),
                                         emb_t[c][:].bitcast(F32))
                    nc.tensor.matmul(rsq_ps[:], ones_bf[:], sq[:],
                                     start=(c == 0), stop=(c == NCH - 1))
                s_row = tiny.tile([1, S], F32, tag="srow")
                nc.scalar.activation(out=s_row[:], in_=rsq_ps[:], func=AFT.Sqrt,
                                     bias=0.0, scale=1.0)
                r_row = tiny.tile([1, S], F32, tag="rrow")
                nc.vector.reciprocal(out=r_row[:], in_=s_row[:])
                nc.sync.dma_start(out=rout_d[b], in_=r_row[:])
                nc.sync.dma_start(out=w2_s[b : b + 1, :], in_=r_row[:])
                W2 = w2pool.tile([128, S], F32, tag="w2")
                nc.sync.dma_start(out=W2[:], in_=w2_s[b : b + 1, :].to_broadcast((128, S)))

                # ---- Q/K projections (d-outer over 4 PSUM banks, 2 rounds each)
                qt_t: list = []
                kt_t: list = []
                for w_t, bc_t, outlist in ((wq_t, bqc_t, qt_t), (wk_t, bkc_t, kt_t)):
                    for rnd in range(2):
                        es = range(rnd * 4, rnd * 4 + 4)
                        pps = [psP.tile([128, S], F32, tag="pp", name=f"pp{rnd}_{j}") for j, _ in enumerate(es)]
                        for c in range(NCH):
                            for j, e in enumerate(es):
                                nc.tensor.matmul(
                                    pps[j][:], w_t[c][:, e * 128 : (e + 1) * 128],
                                    emb_t[c][:], start=(c == 0), stop=(c == NCH - 1))
                        for j, e in enumerate(es):
                            qt = qkpool.tile([128, S], F32R, tag="qkt")
                            nc.scalar.activation(out=qt[:], in_=pps[j][:],
                                                 func=AFT.Identity,
                                                 bias=bc_t[:, e : e + 1], scale=1.0)
                            outlist.append(qt)

                # ---- logits chunks: L = QT.T @ KT + mask rows; per-chunk max
                mx = tiny.tile([128, NIC], F32, tag="mx")
                L_ps = []
                for ic in range(NIC):
                    Lp = psL.tile([128, S], F32, tag="Lps")
                    for e in range(NCH):
                        nc.tensor.matmul(Lp[:], qt_t[e][:, ic * 128 : (ic + 1) * 128],
                                         kt_t[e][:], start=(e == 0), stop=False)
                    nc.tensor.matmul(Lp[:], lr_t[:, ic * 128 : (ic + 1) * 128],
                                     rr_t[:], start=False, stop=True)
                    nc.vector.reduce_max(mx[:, ic : ic + 1], Lp[:], axis=AX.X)
                    L_ps.append(Lp)

                # ---- gram chunks -> Gw = |G| * r_j
                gw_t = []
                for ic in range(NIC):
                    Gp = psP.tile([128, S], F32, tag="pp")
                    for c in range(NCH):
                        nc.tensor.matmul(Gp[:], emb_t[c][:, ic * 128 : (ic + 1) * 128],
                                         emb_t[c][:], start=(c == 0), stop=(c == NCH - 1))
                    ga = gapool.tile([128, S], F32, tag="ga")
                    nc.scalar.activation(out=ga[:], in_=Gp[:], func=AFT.Abs,
                                         bias=0.0, scale=1.0)
                    gw = gwpool.tile([128, S], F32, tag="gw")
                    nc.vector.tensor_mul(gw[:], ga[:], W2[:])
                    gw_t.append(gw)

                # ---- global masked max -> -M broadcast to [128,1]
                gmax = tiny.tile([128, 1], F32, tag="gmax")
                nc.vector.reduce_max(gmax[:], mx[:], axis=AX.X)
                grow = tiny.tile([1, 128], F32, tag="grow")
                nc.sync.dma_start(out=grow[0:1, :], in_=gmax[:, 0:1])
                negm = tiny.tile([1, 1], F32, tag="negm")
                nc.vector.reduce_max(negm[:], grow[:], axis=AX.X, negate=True)
                nc.sync.dma_start(out=negm_s[b : b + 1, :], in_=negm[:])
                negm128 = tiny.tile([128, 1], F32, tag="negm128")
                nc.sync.dma_start(out=negm128[:],
                                  in_=negm_s[b : b + 1, :].to_broadcast((128, 1)))

                # ---- exp + fused weighted reductions
                zcols = tiny.tile([128, NIC], F32, tag="zc")
                wcols = tiny.tile([128, NIC], F32, tag="wc")
                for ic in range(NIC):
                    E = Epool.tile([128, S], F32, tag="E")
                    nc.scalar.activation(out=E[:], in_=L_ps[ic][:], func=AFT.Exp,
                                         bias=negm128[:], scale=1.0,
                                         accum_out=zcols[:, ic : ic + 1])
                    scr = scrpool.tile([128, S], F32, tag="scr")
                    nc.vector.scalar_tensor_tensor(
                        out=scr[:], in0=gw_t[ic][:], scalar=1.0, in1=E[:],
                        op0=ALU.mult, op1=ALU.mult,
                        accum_out=wcols[:, ic : ic + 1])

                nc.sync.dma_start(out=zw_d[b, 0], in_=zcols[:])
                nc.sync.dma_start(out=zw_d[b, 1], in_=wcols[:])

    nc.compile()
    _built = nc
    return nc


def kernel(embeddings, Wq, bq, Wk, bk, attention_masks, token_type_ids):
    global LAST_RESULTS
    nc = _build()

    embeddings = np.ascontiguousarray(np.asarray(embeddings, dtype=np.float32))
    Wq = np.asarray(Wq, dtype=np.float32)
    Wk = np.asarray(Wk, dtype=np.float32)
    bq = np.asarray(bq, dtype=np.float32)
    bk = np.asarray(bk, dtype=np.float32)
    am = np.asarray(attention_masks)
    tt = np.asarray(token_type_ids)

    # host-side layout + fp32r rounding
    embT = _to_fp32r(embeddings.transpose(0, 2, 1))          # [B, D, S]
    embT = embT.reshape(B, NCH, 128, S)
    wqT = _to_fp32r(Wq.T).reshape(NCH, 128, D)               # wqT[d,e] = Wq[e,d]
    wkT = _to_fp32r(Wk.T).reshape(NCH, 128, D)
    bqc = np.ascontiguousarray(bq.reshape(NCH, 128).T)       # [128, NCH]
    bkc = np.ascontiguousarray(bk.reshape(NCH, 128).T)

    tok = am == 1
    m0 = tok & (tt == 0)                                     # [B, S] bool
    m1 = tok & (tt == 1)
    m0neg = np.where(m0, np.float32(0.0), NEG).astype(np.float32)
    m1neg = np.where(m1, np.float32(0.0), NEG).astype(np.float32)
    ones_row = np.ones((B, 1, S), np.float32)
    lrows = _to_fp32r(np.concatenate([m0neg[:, None, :], ones_row], axis=1))  # [B,2,S]
    rrows = _to_fp32r(np.concatenate([ones_row, m1neg[:, None, :]], axis=1))  # [B,2,S]

    in_maps = []
    for i in range(NCORES):
        sl = slice(i * BPC, (i + 1) * BPC)
        in_maps.append({
            "embT": np.ascontiguousarray(embT[sl]),
            "wqT": wqT, "wkT": wkT, "bqc": bqc, "bkc": bkc,
            "lrows": np.ascontiguousarray(lrows[sl]),
            "rrows": np.ascontiguousarray(rrows[sl]),
        })

    res = run_bass_kernel_spmd(nc, in_maps, core_ids=list(range(NCORES)),
                               trace=PROFILE)
    LAST_RESULTS = res

    valid = m0.any(axis=1) & m1.any(axis=1)
    cs = np.zeros(B, np.float64)
    for i in range(NCORES):
        for j in range(BPC):
            b = i * BPC + j
            if not valid[b]:
                continue
            zcols = res.results[i]["zw"][j, 0].astype(np.float64)   # [128, NIC]
            wcols = res.results[i]["zw"][j, 1].astype(np.float64)
            r = res.results[i]["rout"][j].astype(np.float64)        # [S]
            ri = r.reshape(NIC, 128).T                              # [128, NIC]
            z = zcols.sum()
            w = (wcols * ri).sum()
            cs[b] = w / (z + 1e-30)
    return cs.astype(np.float32)


# revision 6
# speedup vs baseline: 1.0995x; 1.0995x over previous
"""Trainium2 Bass kernel for nn_CESAR_24309514895978 (ragged_sequence).

Math (per batch b):
  m0 = (attention_masks==1)&(token_type_ids==0); m1 = (attention_masks==1)&(token_type_ids==1)
  score[i,j] = |emb_n[i]. emb_n[j]|  (L2-normalized embeddings)
  logits[i,j] = (emb@Wq.T+bq)[i] . (emb@Wk.T+bk)[j]
  cs[b] = sum_{valid ij} softmax_flat(logits | pair_mask)[i,j] * score[i,j]

Device computes, per batch (data-parallel: 2 batches per core x 8 cores):
  - rsq[j] = sum_d emb[j,d]^2 via bf16 squares + ones-matmul; r = 1/sqrt(rsq)
  - QT/KT = Wq/Wk projections (fp32r matmuls, PSUM fp32, bias via ACT)
  - L[i,j] = QT.T@KT + (-1e30 masks via a K=2 static matmul row-pair)
  - M = masked max (DVE reduce + tiny transpose DMA + DRAM-roundtrip broadcast)
  - E = exp(L - M)  (ACT, accum_out -> Z partial sums)
  - W partials = sum_j E * |G| * r_j  (G = gram matmul; stt fused mult/mult/accum)
Host: r_i scaling + final sums + W/Z division (tiny), plus input layout/rounding.
"""
import numpy as np

import concourse.bass_isa as bass_isa
import concourse.tile as tile
from concourse import bacc, mybir
from concourse.bass_utils import run_bass_kernel_spmd

B, S, D = 16, 512, 1024
NCORES = 8
BPC = B // NCORES          # batches per core
NCH = D // 128             # 8 contraction chunks
NIC = S // 128             # 4 i-chunks
NEG = np.float32(-1e30)

F32 = mybir.dt.float32
F32R = mybir.dt.float32r
BF16 = mybir.dt.bfloat16
AFT = mybir.ActivationFunctionType
ALU = mybir.AluOpType
AX = mybir.AxisListType

PROFILE = False            # set True (e.g. from test.py) to capture NTFF profile
LAST_RESULTS = None        # BassKernelResults of the last run (for test.py)

_built = None


def _to_fp32r(x: np.ndarray) -> np.ndarray:
    """Round fp32 -> fp32r encoding (RNE to 11 explicit mantissa bits)."""
    u = np.ascontiguousarray(x, dtype=np.float32).view(np.uint32).astype(np.uint64)
    u = (u + 0x7FF + ((u >> 12) & 1)) & np.uint64(0xFFFFF000)
    return u.astype(np.uint32).view(np.float32)


def _build():
    global _built
    if _built is not None:
        return _built

    nc = bacc.Bacc("TRN2", target_bir_lowering=False, debug=False)

    embT_d = nc.dram_tensor("embT", [BPC, NCH, 128, S], F32R, kind="ExternalInput").ap()
    wqT_d = nc.dram_tensor("wqT", [NCH, 128, D], F32R, kind="ExternalInput").ap()
    wkT_d = nc.dram_tensor("wkT", [NCH, 128, D], F32R, kind="ExternalInput").ap()
    bqc_d = nc.dram_tensor("bqc", [128, NCH], F32, kind="ExternalInput").ap()
    bkc_d = nc.dram_tensor("bkc", [128, NCH], F32, kind="ExternalInput").ap()
    lrows_d = nc.dram_tensor("lrows", [BPC, 2, S], F32R, kind="ExternalInput").ap()
    rrows_d = nc.dram_tensor("rrows", [BPC, 2, S], F32R, kind="ExternalInput").ap()

    zw_d = nc.dram_tensor("zw", [BPC, 2, 128, NIC], F32, kind="ExternalOutput").ap()
    rout_d = nc.dram_tensor("rout", [BPC, S], F32, kind="ExternalOutput").ap()


    with tile.TileContext(nc) as tc:
        with (
            tc.tile_pool(name="wpool", bufs=16) as wpool,
            tc.tile_pool(name="spool", bufs=1) as spool,
            tc.tile_pool(name="epool", bufs=16) as epool,
            tc.tile_pool(name="sqpool", bufs=3) as sqpool,
            tc.tile_pool(name="qkpool", bufs=16) as qkpool,
            tc.tile_pool(name="w2pool", bufs=2) as w2pool,
            tc.tile_pool(name="gapool", bufs=2) as gapool,
            tc.tile_pool(name="gwpool", bufs=4) as gwpool,
            tc.tile_pool(name="Epool", bufs=2) as Epool,
            tc.tile_pool(name="scrpool", bufs=1) as scrpool,
            tc.tile_pool(name="tiny", bufs=2) as tiny,
            tc.tile_pool(name="lrpool", bufs=2) as lrpool,
            tc.tile_pool(name="psP", bufs=4, space="PSUM") as psP,
            tc.tile_pool(name="psL", bufs=4, space="PSUM") as psL,
        ):
            # DMA emission order matters for startup: batch-0 embeddings
            # interleaved with Wq chunks first (first PE work needs them),
            # Wk afterwards (K proj starts ~30us in).
            emb_all = [[None] * NCH for _ in range(BPC)]
            wq_t, wk_t = [], []
            for c in range(NCH):
                t = epool.tile([128, S], F32R, tag="emb", name=f"emb0_{c}")
                nc.sync.dma_start(out=t[:], in_=embT_d[0, c])
                emb_all[0][c] = t
                t = wpool.tile([128, D], F32R, tag="w", name=f"wq_{c}")
                nc.sync.dma_start(out=t[:], in_=wqT_d[c])
                wq_t.append(t)
            for c in range(NCH):
                t = wpool.tile([128, D], F32R, tag="w", name=f"wk_{c}")
                nc.sync.dma_start(out=t[:], in_=wkT_d[c])
                wk_t.append(t)
            bqc_t = spool.tile([128, NCH], F32, tag="bqc")
            nc.sync.dma_start(out=bqc_t[:], in_=bqc_d)
            bkc_t = spool.tile([128, NCH], F32, tag="bkc")
            nc.sync.dma_start(out=bkc_t[:], in_=bkc_d)
            ones_bf = spool.tile([128, 1], BF16, tag="ones_bf")
            nc.vector.memset(ones_bf[:], 1.0)

            for b in range(BPC):
                # ---- load inputs for this batch
                if b > 0:
                    for c in range(NCH):
                        t = epool.tile([128, S], F32R, tag="emb", name=f"emb{b}_{c}")
                        nc.sync.dma_start(out=t[:], in_=embT_d[b, c])
                        emb_all[b][c] = t
                emb_t = emb_all[b]
                lr_t = lrpool.tile([2, S], F32R, tag="lr")
                nc.sync.dma_start(out=lr_t[:], in_=lrows_d[b])
                rr_t = lrpool.tile([2, S], F32R, tag="rr")
                nc.sync.dma_start(out=rr_t[:], in_=rrows_d[b])

                # ---- rsq / r / W2 (row of 1/||e_j||, replicated to 128 partitions)
                rsq_ps = psL.tile([1, S], F32, tag="Lps")
                for c in range(NCH):
                    sq = sqpool.tile([128, S], BF16, tag="sq")
                    nc.vector.tensor_mul(sq[:], emb_t[c][:].bitcast(F32),
                                         emb_t[c][:].bitcast(F32))
                    nc.tensor.matmul(rsq_ps[:], ones_bf[:], sq[:],
                                     start=(c == 0), stop=(c == NCH - 1))
                s_row = tiny.tile([1, S], F32, tag="srow")
                nc.scalar.activation(out=s_row[:], in_=rsq_ps[:], func=AFT.Sqrt,
                                     bias=0.0, scale=1.0)
                r_row = tiny.tile([1, S], F32, tag="rrow")
                nc.vector.reciprocal(out=r_row[:], in_=s_row[:])
                nc.sync.dma_start(out=rout_d[b], in_=r_row[:])
                W2 = w2pool.tile([128, S], F32, tag="w2")
                nc.gpsimd.partition_broadcast(W2[:], r_row[0:1, :], channels=128)

                # ---- Q/K projections (d-outer over 4 PSUM banks, 2 rounds each)
                qt_t: list = []
                kt_t: list = []
                for w_t, bc_t, outlist in ((wq_t, bqc_t, qt_t), (wk_t, bkc_t, kt_t)):
                    for rnd in range(2):
                        es = range(rnd * 4, rnd * 4 + 4)
                        pps = [psP.tile([128, S], F32, tag="pp", name=f"pp{rnd}_{j}") for j, _ in enumerate(es)]
                        for c in range(NCH):
                            for j, e in enumerate(es):
                                nc.tensor.matmul(
                                    pps[j][:], w_t[c][:, e * 128 : (e + 1) * 128],
                                    emb_t[c][:], start=(c == 0), stop=(c == NCH - 1))
                        for j, e in enumerate(es):
                            qt = qkpool.tile([128, S], F32R, tag="qkt")
                            nc.scalar.activation(out=qt[:], in_=pps[j][:],
                                                 func=AFT.Identity,
                                                 bias=bc_t[:, e : e + 1], scale=1.0)
                            outlist.append(qt)

                # ---- logits chunks: L = QT.T @ KT + mask rows; per-chunk max
                mx = tiny.tile([128, NIC], F32, tag="mx")
                L_ps = []
                for ic in range(NIC):
                    Lp = psL.tile([128, S], F32, tag="Lps")
                    for e in range(NCH):
                        nc.tensor.matmul(Lp[:], qt_t[e][:, ic * 128 : (ic + 1) * 128],
                                         kt_t[e][:], start=(e == 0), stop=False)
                    nc.tensor.matmul(Lp[:], lr_t[:, ic * 128 : (ic + 1) * 128],
                                     rr_t[:], start=False, stop=True)
                    nc.vector.reduce_max(mx[:, ic : ic + 1], Lp[:], axis=AX.X)
                    L_ps.append(Lp)

                # ---- global masked max -> -M broadcast to [128,1]
                # (gpsimd all-reduce across partitions, then free-dim max+negate)
                par = tiny.tile([128, NIC], F32, tag="par")
                nc.gpsimd.partition_all_reduce(par[:], mx[:], channels=128,
                                               reduce_op=bass_isa.ReduceOp.max)
                negm128 = tiny.tile([128, 1], F32, tag="negm128")
                nc.vector.reduce_max(negm128[:], par[:], axis=AX.X, negate=True)

                # ---- gram chunks -> Gw = |G| * r_j
                gw_t = []
                for ic in range(NIC):
                    Gp = psP.tile([128, S], F32, tag="pp")
                    for c in range(NCH):
                        nc.tensor.matmul(Gp[:], emb_t[c][:, ic * 128 : (ic + 1) * 128],
                                         emb_t[c][:], start=(c == 0), stop=(c == NCH - 1))
                    ga = gapool.tile([128, S], F32, tag="ga")
                    nc.scalar.activation(out=ga[:], in_=Gp[:], func=AFT.Abs,
                                         bias=0.0, scale=1.0)
                    gw = gwpool.tile([128, S], F32, tag="gw")
                    nc.vector.tensor_mul(gw[:], ga[:], W2[:])
                    gw_t.append(gw)

                # ---- exp + fused weighted reductions
                zcols = tiny.tile([128, NIC], F32, tag="zc")
                wcols = tiny.tile([128, NIC], F32, tag="wc")
                for ic in range(NIC):
                    E = Epool.tile([128, S], F32, tag="E")
                    nc.scalar.activation(out=E[:], in_=L_ps[ic][:], func=AFT.Exp,
                                         bias=negm128[:], scale=1.0,
                                         accum_out=zcols[:, ic : ic + 1])
                    scr = scrpool.tile([128, S], F32, tag="scr")
                    nc.vector.scalar_tensor_tensor(
                        out=scr[:], in0=gw_t[ic][:], scalar=1.0, in1=E[:],
                        op0=ALU.mult, op1=ALU.mult,
                        accum_out=wcols[:, ic : ic + 1])

                nc.sync.dma_start(out=zw_d[b, 0], in_=zcols[:])
                nc.sync.dma_start(out=zw_d[b, 1], in_=wcols[:])

    nc.compile()
    _built = nc
    return nc


def kernel(embeddings, Wq, bq, Wk, bk, attention_masks, token_type_ids):
    global LAST_RESULTS
    nc = _build()

    embeddings = np.ascontiguousarray(np.asarray(embeddings, dtype=np.float32))
    Wq = np.asarray(Wq, dtype=np.float32)
    Wk = np.asarray(Wk, dtype=np.float32)
    bq = np.asarray(bq, dtype=np.float32)
    bk = np.asarray(bk, dtype=np.float32)
    am = np.asarray(attention_masks)
    tt = np.asarray(token_type_ids)

    # host-side layout + fp32r rounding
    embT = _to_fp32r(embeddings.transpose(0, 2, 1))          # [B, D, S]
    embT = embT.reshape(B, NCH, 128, S)
    wqT = _to_fp32r(Wq.T).reshape(NCH, 128, D)               # wqT[d,e] = Wq[e,d]
    wkT = _to_fp32r(Wk.T).reshape(NCH, 128, D)
    bqc = np.ascontiguousarray(bq.reshape(NCH, 128).T)       # [128, NCH]
    bkc = np.ascontiguousarray(bk.reshape(NCH, 128).T)

    tok = am == 1
    m0 = tok & (tt == 0)                                     # [B, S] bool
    m1 = tok & (tt == 1)
    m0neg = np.where(m0, np.float32(0.0), NEG).astype(np.float32)
    m1neg = np.where(m1, np.float32(0.0), NEG).astype(np.float32)
    ones_row = np.ones((B, 1, S), np.float32)
    lrows = _to_fp32r(np.concatenate([m0neg[:, None, :], ones_row], axis=1))  # [B,2,S]
    rrows = _to_fp32r(np.concatenate([ones_row, m1neg[:, None, :]], axis=1))  # [B,2,S]

    in_maps = []
    for i in range(NCORES):
        sl = slice(i * BPC, (i + 1) * BPC)
        in_maps.append({
            "embT": np.ascontiguousarray(embT[sl]),
            "wqT": wqT, "wkT": wkT, "bqc": bqc, "bkc": bkc,
            "lrows": np.ascontiguousarray(lrows[sl]),
            "rrows": np.ascontiguousarray(rrows[sl]),
        })

    res = run_bass_kernel_spmd(nc, in_maps, core_ids=list(range(NCORES)),
                               trace=PROFILE)
    LAST_RESULTS = res

    valid = m0.any(axis=1) & m1.any(axis=1)
    cs = np.zeros(B, np.float64)
    for i in range(NCORES):
        for j in range(BPC):
            b = i * BPC + j
            if not valid[b]:
                continue
            zcols = res.results[i]["zw"][j, 0].astype(np.float64)   # [128, NIC]
            wcols = res.results[i]["zw"][j, 1].astype(np.float64)
            r = res.results[i]["rout"][j].astype(np.float64)        # [S]
            ri = r.reshape(NIC, 128).T                              # [128, NIC]
            z = zcols.sum()
            w = (wcols * ri).sum()
            cs[b] = w / (z + 1e-30)
    return cs.astype(np.float32)
